# revision 1
# baseline (speedup 1.0000x reference)
"""Trainium2 Bass kernel for nn_DilatedAttentionBlock_attention.

Per-core work (data-parallel over batch, 8 cores):
  x [C=256, L=2048] -> QKV MLPs -> 4-head attention with Lipschitz score
  rescale -> out-proj -> LayerNorm -> ELU + residual -> out [C, L].

Everything stays in channel-major ("transposed") [C, L] layout, which is the
native layout of x_in, so weights act as natural lhsT operands and no input
or output transposes are needed.  Scores are computed directly transposed
(S^T[k, q]) so the softmax exp on the scalar engine doubles as the
PSUM->SBUF copy and the attention matrix never needs transposing for AV.
Head pairs run concurrently on the PE via disjoint row groups (contract
dim is 64).

Key algebraic tricks (exact, up to float rounding):
  - row_norm^2[q] = Q[q]^T (K^T K) Q[q]: computed via a tiny 64x64 Gram
    matrix instead of reducing the 2048x2048 score matrix.  alpha[q] =
    1/sqrt(t[q]) is folded into Q before the score matmul.  (The
    reference's EPS=1e-12 inside the rescale is negligible: sqrt(t) is
    O(40) for this data distribution; dropped.)
  - softmax denominator: V gets a ones-column appended (M=65 AV matmul),
    so row 64 of the AV accumulator is sum_k exp(s); the division happens
    on the [64, L] attention output, not on the [L, L] attention matrix.
Matmuls run in float32r (full PE column rate; ~1e-4 relative accuracy,
verified on hardware) with fp32 PSUM accumulation.

SBUF note: tile tags are deliberately shared across phases (e.g. LN/ELU
temporaries reuse the K-transpose and Q-tilde slots) to stay inside the
192KB/partition budget; PSUM uses exactly four 2-bank tags.
"""

import numpy as np

import concourse.bacc as bacc
import concourse.bass as bass
import concourse.mybir as mybir
import concourse.tile as tile
from concourse.bass_utils import run_bass_kernel_spmd
from concourse.masks import make_identity

B, C, L, H, HD = 8, 256, 2048, 4, 64
P = 128
NCORES = 8
LH = L // 2  # 1024, attention q-half width
FP32 = mybir.dt.float32
FP32R = mybir.dt.float32r
AF = mybir.ActivationFunctionType
OP = mybir.AluOpType

W_NAMES = ["q_w1", "q_w2", "k_w1", "k_w2", "v_w1", "v_w2", "o_w"]
B_NAMES = ["q_b1", "q_b2", "k_b1", "k_b2", "v_b1", "v_b2", "o_b", "ln_g", "ln_b"]

LN_EPS = 1e-5
INV_C = 1.0 / C
BF16 = mybir.dt.bfloat16
# score/AV matmul dtype: bf16 runs the PE at full rate with fast weight
# load and allows N=1024 moving operands (half the instruction count).
SDT = BF16
NMM = 512

PS_TAGS = ["pA0", "pA1", "pB0", "pB1"]


class Ctx:
    """Holds pools + round-robin psum tag allocation."""

    def __init__(self, nc, tc, pools):
        self.nc = nc
        self.tc = tc
        (self.consts, self.wpool, self.stage, self.gelu, self.qkv, self.attp,
         self.ps, self.rowp, self.bcp, self.dramp) = pools
        self._ps_i = 0

    def ps_tile(self, shape, name):
        tag = PS_TAGS[self._ps_i % 4]
        self._ps_i += 1
        return self.ps.tile(shape, FP32, tag=tag, name=name, bufs=1)


def _linear_T(cx, w_sb, rhs_tiles, out_tiles, act_fn, bias_sb):
    """out^T[m, l] = act(sum_k w[k, m] * rhs^T[k, l] + bias[m]).

    w_sb [P, 2, C] fp32r; rhs_tiles: 2 tiles [P, L] fp32r (contraction
    k-outer); out_tiles: 2 tiles [P, L].  PSUM in [P, LH] chunks.
    """
    nc = cx.nc
    for m in range(2):
        for lh in range(2):
            acc = cx.ps_tile([P, LH], f"lin_acc_{m}_{lh}")
            for lg in range(2):
                psl = slice(lg * 512, (lg + 1) * 512)
                gsl = slice(lh * LH + lg * 512, lh * LH + (lg + 1) * 512)
                for k in range(2):
                    nc.tensor.matmul(
                        acc[:, psl],
                        w_sb[:, k, m * P : (m + 1) * P],
                        rhs_tiles[k][:, gsl],
                        start=(k == 0),
                        stop=(k == 1),
                    )
            osl = slice(lh * LH, (lh + 1) * LH)
            if act_fn is not None:
                cx.nc.scalar.activation(
                    out_tiles[m][:, osl], acc[:], act_fn,
                    bias=bias_sb[:, m : m + 1],
                )
            else:
                nc.vector.tensor_scalar_add(
                    out_tiles[m][:, osl], acc[:], bias_sb[:, m : m + 1]
                )


def _bcast(cx, out_ap, row_ap, name):
    """Broadcast a [1, N] SBUF row across partitions via a DRAM bounce
    (SBUF->SBUF DMA cannot have a zero partition step, DRAM->SBUF can)."""
    nc = cx.nc
    n = row_ap.shape[-1]
    d = cx.dramp.tile([1, n], FP32, tag="drow", name=name, bufs=2)
    nc.sync.dma_start(out=d[:], in_=row_ap)
    nc.sync.dma_start(out=out_ap, in_=d.to_broadcast(list(out_ap.shape)))


def _build_body(cx, phases=4):
    nc = cx.nc
    x_in, out = nc.x_in_t, nc.out_t

    def _dump(tiles):
        for m, t in enumerate(tiles):
            v = t.bitcast(FP32)
            nc.sync.dma_start(out[m * P : (m + 1) * P, 0 : v.shape[-1]], v[:])

    # ---- constants (fp32r written via compute-engine rounding copies) ----
    ident_st = cx.consts.tile([P, P], FP32)
    make_identity(nc, ident_st)
    ident = cx.consts.tile([P, P], FP32R)
    nc.vector.tensor_copy(ident[:], ident_st[:])
    ones_st = cx.consts.tile([P, 32], FP32)
    nc.vector.memset(ones_st[:], 1.0)
    ones_64_1 = cx.consts.tile([64, 1], FP32R)
    nc.vector.tensor_copy(ones_64_1[:], ones_st[0:64, 0:1])
    invc_st = cx.consts.tile([P, 1], FP32)
    nc.vector.memset(invc_st[:], INV_C)
    invc_128 = cx.consts.tile([P, 1], FP32R)
    nc.vector.tensor_copy(invc_128[:], invc_st[:])
    eps_sb = cx.consts.tile([1, 1], FP32)
    nc.vector.memset(eps_sb[:], LN_EPS)

    # ---- x load + round ----
    x_re = x_in.rearrange("(ko ki) l -> ki ko l", ki=P)
    xst = cx.stage.tile([P, 2, L], FP32, tag="x_st")
    xr = cx.stage.tile([P, 2, L], FP32R, tag="xr")
    for ko in range(2):
        for xh in range(2):
            xsl = slice(xh * LH, (xh + 1) * LH)
            nc.sync.dma_start(xst[:, ko, xsl], x_re[:, ko, xsl])
            nc.vector.tensor_copy(xr[:, ko, xsl], xst[:, ko, xsl])

    b_sb = {}
    for name in B_NAMES:
        t = cx.consts.tile([P, 2], FP32, name="b_" + name)
        nc.sync.dma_start(
            t[:], getattr(nc, name + "_t").rearrange("(mo mi) -> mi mo", mi=P)
        )
        b_sb[name] = t


    def load_w(name, tag):
        st = cx.wpool.tile([P, 2, C], FP32, tag="w_stage", bufs=2,
                           name=f"wst_{name}")
        nc.sync.dma_start(
            st[:], getattr(nc, name + "_t").rearrange("(ko ki) m -> ki ko m", ki=P)
        )
        wr = cx.wpool.tile([P, 2, C], FP32R, tag=tag, name=f"w_{name}", bufs=1)
        nc.vector.tensor_copy(wr[:], st[:])
        return wr

    # ---- QKV projections ----
    proj = {}
    for p in ("q", "k", "v"):
        w1 = load_w(p + "_w1", "wA")
        w2 = load_w(p + "_w2", "wB")
        g = [cx.gelu.tile([P, L], FP32R, tag=f"g{m}", name=f"g_{p}{m}", bufs=1)
             for m in range(2)]
        _linear_T(cx, w1, [xr[:, 0], xr[:, 1]], g, AF.Gelu, b_sb[p + "_b1"])
        o = [cx.qkv.tile([P, L], FP32R, tag=f"{p}{m}", name=f"{p}_sb{m}", bufs=1)
             for m in range(2)]
        _linear_T(cx, w2, g, o, None, b_sb[p + "_b2"])
        proj[p] = o
    q_sb, k_sb, v_sb = proj["q"], proj["k"], proj["v"]
    if phases == 1:
        _dump(q_sb)
        return
    ow_sb = load_w("o_w", "w_ow")

    # ---- PE transposes: K -> k_t (for Gram), V -> v_t (ones-augmented) ----
    k_t, v_t = [], []
    for ct in range(2):
        kt_tile = cx.attp.tile([P, 16, P], FP32R, tag=f"k_t{ct}",
                               name=f"k_t{ct}", bufs=1)
        vt_tile = cx.attp.tile([P, 16, 130], SDT, tag=f"v_t{ct}",
                               name=f"v_t{ct}", bufs=1)
        nc.vector.tensor_copy(
            vt_tile.rearrange("p l (h c) -> p l h c", h=2)[:, :, :, 64:65],
            ones_st.rearrange("p (l h c) -> p l h c", l=16, h=2),
        )
        for lt0 in range(0, 16, 4):
            trk = cx.ps_tile([P, 512], f"trk_{ct}_{lt0}")
            for j in range(4):
                nc.tensor.transpose(
                    trk.bitcast(FP32R)[:, j * P : (j + 1) * P],
                    k_sb[ct][:, (lt0 + j) * P : (lt0 + j + 1) * P],
                    ident[:],
                )
            nc.vector.tensor_copy(
                kt_tile[:, lt0 : lt0 + 4, :],
                trk.bitcast(FP32R).rearrange("p (l c) -> p l c", l=4),
            )
            trv = cx.ps_tile([P, 512], f"trv_{ct}_{lt0}")
            for j in range(4):
                nc.tensor.transpose(
                    trv.bitcast(FP32R)[:, j * P : (j + 1) * P],
                    v_sb[ct][:, (lt0 + j) * P : (lt0 + j + 1) * P],
                    ident[:],
                )
            nc.vector.tensor_copy(
                vt_tile[:, lt0 : lt0 + 4, :]
                .rearrange("p l (h c) -> p l h c", h=2)[:, :, :, 0:64],
                trv.bitcast(FP32R).rearrange("p (l h c) -> p l h c", l=4, h=2),
            )
        k_t.append(kt_tile)
        v_t.append(vt_tile)

    # ---- Gram matrices -> alpha -> Q-tilde ----
    # Per channel-tile (head pair): one full 128x128 Gram matmul series
    # (the off-diagonal cross-head blocks are discarded), then a
    # block-diagonal G so GQ/QGQ/t/ln/exp/broadcast all run pair-wide:
    #   t[h, q] = sum_d Q_h[d, q] * (G_h Q_h)[d, q],  alpha = exp(-0.5 ln t)
    sel_st = cx.consts.tile([P, 2], FP32)
    nc.vector.memset(sel_st[:], 0.0)
    nc.vector.memset(sel_st[0:64, 0:1], 1.0)
    nc.vector.memset(sel_st[64:128, 1:2], 1.0)
    sel2 = cx.consts.tile([P, 2], FP32R)
    nc.vector.tensor_copy(sel2[:], sel_st[:])
    gz_st = cx.consts.tile([P, P], FP32)
    nc.vector.memset(gz_st[:], 0.0)

    qt_sb = []
    for ct in range(2):
        abc = cx.bcp.tile([P, L], FP32, tag="bc", name=f"abc{ct}", bufs=2)
        g_ps = cx.ps_tile([P, P], f"g_ps{ct}")
        for kt in range(16):
            nc.tensor.matmul(
                g_ps[:], k_t[ct][:, kt, :], k_t[ct][:, kt, :],
                start=(kt == 0), stop=(kt == 15),
            )
        g_pair = cx.rowp.tile([P, P], FP32R, tag="gram", name=f"g_pair{ct}",
                              bufs=1)
        nc.vector.tensor_copy(g_pair[:], gz_st[:])
        for ho in range(2):
            hsl = slice(64 * ho, 64 * ho + 64)
            nc.vector.tensor_copy(g_pair[hsl, hsl], g_ps.bitcast(FP32R)[hsl, hsl])
        qgq = cx.rowp.tile([P, L], FP32R, tag="qgq", name=f"qgq{ct}", bufs=1)
        for lh in range(2):
            lsl = slice(lh * LH, (lh + 1) * LH)
            gq_ps = cx.ps_tile([P, LH], f"gq_ps{ct}{lh}")
            for lg in range(2):
                psl = slice(lg * 512, (lg + 1) * 512)
                gsl = slice(lh * LH + lg * 512, lh * LH + (lg + 1) * 512)
                nc.tensor.matmul(gq_ps[:, psl], g_pair[:], q_sb[ct][:, gsl],
                                 start=True, stop=True)
            nc.vector.tensor_tensor(out=qgq[:, lsl], in0=q_sb[ct][:, lsl],
                                    in1=gq_ps[:], op=OP.mult)
        lnt = cx.rowp.tile([2, L], FP32, tag="rowA", name=f"lnt{ct}", bufs=1)
        for lh in range(2):
            lsl = slice(lh * LH, (lh + 1) * LH)
            t_ps = cx.ps_tile([2, LH], f"t_ps{ct}{lh}")
            for lg in range(2):
                psl = slice(lg * 512, (lg + 1) * 512)
                nc.tensor.matmul(t_ps[:, psl], sel2[:],
                                 qgq[:, lh * LH + lg * 512 : lh * LH + (lg + 1) * 512],
                                 start=True, stop=True)
            nc.scalar.activation(lnt[:, lsl], t_ps[:], AF.Ln)
        a_pair = cx.rowp.tile([2, L], FP32, tag="rowB", name=f"apair{ct}",
                              bufs=1)
        nc.scalar.activation(a_pair[:], lnt[:], AF.Exp, scale=-0.5)
        ad = cx.dramp.tile([2, L], FP32, tag="drow2", name=f"ad{ct}", bufs=2)
        nc.sync.dma_start(out=ad[:], in_=a_pair[:])
        abc_src = bass.AP(tensor=ad.tensor, offset=ad.offset,
                          ap=[[L, 2], [0, 64], [1, L]])
        nc.sync.dma_start(out=abc[:], in_=abc_src)
        qt = cx.gelu.tile([P, L], SDT, tag=f"g{ct}", name=f"qt{ct}", bufs=1)
        nc.vector.tensor_tensor(out=qt[:], in0=q_sb[ct][:], in1=abc[:],
                                op=OP.mult)
        qt_sb.append(qt)
    kb = cx.qkv.tile([P, 2, L], SDT, tag="kb", name="kb", bufs=1)
    for ct in range(2):
        nc.vector.tensor_copy(kb[:, ct, :], k_sb[ct][:])
    if phases == 2:
        _dump(qt_sb)
        return

    # ---- attention + per-half tails ----
    # Emission order matters: engine queues are in-order, so the qh0 tail is
    # emitted after the first qh1 quarter (its PSUM deps are then already
    # satisfied) and tail PSUM tiles use the a-tags, keeping the b-tags
    # attention-only.
    y_sb = [cx.qkv.tile([P, L], FP32R, tag=f"v{ct}", name=f"y{ct}", bufs=1)
            for ct in range(2)]
    z_sb = [cx.qkv.tile([P, L], FP32R, tag=f"q{m}", name=f"z{m}", bufs=1)
            for m in range(2)]

    def quarter(qh, ct, ho):
        """One head, one q-half.  The score accumulator is double-buffered
        on kt parity so exp[kt] overlaps the kt+1 score matmuls, and AV lags
        one kt so it is never exp-gated at the head of the in-order PE
        queue."""
        q0 = qh * LH
        hslice = slice(q0, q0 + LH)
        hsl = slice(64 * ho, 64 * ho + 64)
        b_ps = cx.ps.tile([65, LH], FP32, tag=PS_TAGS[2 + ho],
                          name=f"av{ct}{qh}{ho}", bufs=1)

        def s_mm(kt):
            a = cx.ps.tile([P, LH], FP32, tag=PS_TAGS[kt % 2],
                           name=f"s{ct}{qh}{kt}{ho}", bufs=1)
            for lg in range(LH // NMM):
                psl = slice(lg * NMM, (lg + 1) * NMM)
                nc.tensor.matmul(
                    a[:, psl],
                    kb[hsl, ct, kt * P : (kt + 1) * P],
                    qt_sb[ct][hsl, q0 + lg * NMM : q0 + (lg + 1) * NMM],
                    start=True, stop=True,
                )
            return a

        def av_mm(kt, attn):
            for lg in range(LH // NMM):
                psl = slice(lg * NMM, (lg + 1) * NMM)
                nc.tensor.matmul(
                    b_ps[:, psl],
                    v_t[ct][:, kt, 65 * ho : 65 * ho + 65],
                    attn[:, psl],
                    start=(kt == 0), stop=(kt == 15),
                )

        a_cur = s_mm(0)
        attn_prev = None
        for kt in range(16):
            attn = cx.attp.tile([P, LH], SDT, tag=f"attn{kt % 2}",
                                name=f"at{ct}{qh}{kt}{ho}", bufs=3)
            nc.scalar.activation(attn[:], a_cur[:], AF.Exp)
            if kt < 15:
                a_cur = s_mm(kt + 1)
            if attn_prev is not None:
                av_mm(kt - 1, attn_prev)
            attn_prev = attn
        av_mm(15, attn_prev)

        # drain: pull Y/d out of PSUM promptly, then divide
        invd = cx.rowp.tile([1, LH], FP32, tag="rowA",
                            name=f"invd{ct}{qh}{ho}", bufs=1)
        nc.vector.reciprocal(invd[:], b_ps[64:65, :])
        yc = cx.rowp.tile([64, LH], FP32,
                          tag=("qgq" if ho == 0 else "rowC"),
                          name=f"yc{qh}{ct}{ho}", bufs=1)
        nc.vector.tensor_copy(yc[:], b_ps[0:64, :])
        dbc = cx.bcp.tile([64, LH], FP32, tag="bc",
                          name=f"dbc{ct}{qh}{ho}", bufs=2)
        _bcast(cx, dbc[:], invd[:], f"invd_d{ct}{qh}{ho}")
        nc.vector.tensor_tensor(
            out=y_sb[ct][hsl, hslice], in0=yc[:], in1=dbc[:], op=OP.mult,
        )

    def half_tail(qh):
        q0 = qh * LH
        hslice = slice(q0, q0 + LH)
        for m in range(2):
            acc = cx.ps.tile([P, LH], FP32, tag=PS_TAGS[2 + m],
                             name=f"zacc{qh}{m}", bufs=1)
            for lg in range(2):
                psl = slice(lg * 512, (lg + 1) * 512)
                gsl = slice(q0 + lg * 512, q0 + (lg + 1) * 512)
                for ct in range(2):
                    nc.tensor.matmul(
                        acc[:, psl],
                        ow_sb[:, ct, m * P : (m + 1) * P],
                        y_sb[ct][:, gsl],
                        start=(ct == 0), stop=(ct == 1),
                    )
            nc.vector.tensor_scalar_add(z_sb[m][:, hslice], acc[:],
                                        b_sb["o_b"][:, m : m + 1])
        z2h = [cx.rowp.tile([P, LH], FP32R,
                            tag=("qgq" if m == 0 else "z2b"),
                            name=f"z2_{qh}{m}", bufs=1) for m in range(2)]
        for m in range(2):
            nc.vector.tensor_tensor(out=z2h[m][:], in0=z_sb[m][:, hslice],
                                    in1=z_sb[m][:, hslice], op=OP.mult)
        s1_ps = cx.ps.tile([1, LH], FP32, tag=PS_TAGS[2], name=f"s1_{qh}",
                           bufs=1)
        s2_ps = cx.ps.tile([1, LH], FP32, tag=PS_TAGS[3], name=f"s2_{qh}",
                           bufs=1)
        for lg in range(2):
            psl = slice(lg * 512, (lg + 1) * 512)
            gsl = slice(q0 + lg * 512, q0 + (lg + 1) * 512)
            for m in range(2):
                nc.tensor.matmul(s1_ps[:, psl], invc_128[:], z_sb[m][:, gsl],
                                 start=(m == 0), stop=(m == 1))
            for m in range(2):
                nc.tensor.matmul(s2_ps[:, psl], invc_128[:], z2h[m][:, psl],
                                 start=(m == 0), stop=(m == 1))
        # mean/var row chain ([1, LH] vectors)
        mu = cx.rowp.tile([1, LH], FP32, tag="rowA", name=f"mu{qh}", bufs=1)
        nc.vector.tensor_copy(mu[:], s1_ps[:])
        var = cx.rowp.tile([1, LH], FP32, tag="rowB", name=f"var{qh}", bufs=1)
        nc.vector.tensor_tensor(out=var[:], in0=mu[:], in1=mu[:], op=OP.mult)
        var2 = cx.rowp.tile([1, LH], FP32, tag="rowC", name=f"var2{qh}",
                            bufs=1)
        nc.vector.scalar_tensor_tensor(out=var2[:], in0=s2_ps[:], scalar=0.0,
                                       in1=var[:], op0=OP.add,
                                       op1=OP.subtract)
        lnv = cx.rowp.tile([1, LH], FP32, tag="rowB", name=f"lnv{qh}", bufs=1)
        nc.scalar.activation(lnv[:], var2[:], AF.Ln, bias=eps_sb[:])
        rstd = cx.rowp.tile([1, LH], FP32, tag="rowC", name=f"rstd{qh}",
                            bufs=1)
        nc.scalar.activation(rstd[:], lnv[:], AF.Exp, scale=-0.5)
        mrs = cx.rowp.tile([1, LH], FP32, tag="rowB", name=f"mrs{qh}", bufs=1)
        nc.vector.tensor_tensor(out=mrs[:], in0=mu[:], in1=rstd[:],
                                op=OP.mult)
        ab = cx.stage.tile([P, 2, LH], FP32, tag="x_st", name=f"ab{qh}",
                           bufs=1)
        abd = cx.dramp.tile([2, LH], FP32, tag="drow2", name=f"abd{qh}",
                            bufs=2)
        nc.sync.dma_start(out=abd[0:1, :], in_=rstd[:])
        nc.sync.dma_start(out=abd[1:2, :], in_=mrs[:])
        ab_src = bass.AP(tensor=abd.tensor, offset=abd.offset,
                         ap=[[0, P], [LH, 2], [1, LH]])
        nc.sync.dma_start(out=ab[:], in_=ab_src)
        for m in range(2):
            # reuse the already-consumed y_sb region of this l-half as the
            # LN/ELU workspace (write-after-read; no extra SBUF slot)
            u = y_sb[m][:, hslice]
            nc.vector.tensor_tensor(out=u[:], in0=z_sb[m][:, hslice],
                                    in1=ab[:, 0, :], op=OP.mult)
            nc.vector.tensor_tensor(out=u[:], in0=u[:], in1=ab[:, 1, :],
                                    op=OP.subtract)
            nc.vector.tensor_scalar(
                out=u[:], in0=u[:],
                scalar1=b_sb["ln_g"][:, m : m + 1],
                scalar2=b_sb["ln_b"][:, m : m + 1],
                op0=OP.mult, op1=OP.add,
            )
            # elu(u) + x = relu(u) + exp(min(u,0)) - 1 + x
            neg = cx.attp.tile([P, LH], FP32, tag="k_t0", name=f"neg{qh}{m}",
                               bufs=1)
            nc.vector.tensor_scalar_min(neg[:], u[:], 0.0)
            e = cx.attp.tile([P, LH], FP32, tag="k_t1", name=f"e{qh}{m}",
                             bufs=1)
            nc.scalar.activation(e[:], neg[:], AF.Exp)
            nc.vector.scalar_tensor_tensor(out=u[:], in0=u[:], scalar=0.0,
                                           in1=e[:], op0=OP.max, op1=OP.add)
            nc.vector.scalar_tensor_tensor(out=u[:], in0=u[:], scalar=-1.0,
                                           in1=xr[:, m, hslice], op0=OP.add,
                                           op1=OP.add)
            nc.sync.dma_start(out[m * P : (m + 1) * P, hslice],
                              u.bitcast(FP32)[:])

    if phases == 3:
        for qh in range(2):
            for ct in range(2):
                for ho in range(2):
                    quarter(qh, ct, ho)
        _dump(y_sb)
        return
    for ct in range(2):
        for ho in range(2):
            quarter(0, ct, ho)
    half_tail(0)
    for ct in range(2):
        for ho in range(2):
            quarter(1, ct, ho)
    half_tail(1)

def _steer_act_tables():
    """The act-table-load pass picks the first set containing each
    function, which thrashes natural_log <-> exp_and_others when a kernel
    uses both Ln and Exp.  Empty out the single-function sets so both
    resolve to natural_log_exp_and_others (ids keep their positions)."""
    import concourse.hw_specs as hw_specs
    if getattr(hw_specs, "_act_tables_steered", False):
        return
    orig = hw_specs.get_activation_tables

    def patched(arch):
        t = dict(orig(arch))
        for k in ("natural_log", "exp_and_others", "exp_and_friends"):
            if k in t:
                t[k] = set()
        return t

    hw_specs.get_activation_tables = patched
    bacc.get_activation_tables = patched
    hw_specs._act_tables_steered = True


def build_nc(repeat: int = 1, phases: int = 4):
    _steer_act_tables()
    nc = bacc.Bacc("TRN2", target_bir_lowering=False)
    nc.x_in_t = nc.dram_tensor("x_in", [C, L], FP32, kind="ExternalInput")
    for name in W_NAMES:
        setattr(nc, name + "_t",
                nc.dram_tensor(name, [C, C], FP32, kind="ExternalInput"))
    for name in B_NAMES:
        setattr(nc, name + "_t",
                nc.dram_tensor(name, [C], FP32, kind="ExternalInput"))
    nc.out_t = nc.dram_tensor("out", [C, L], FP32, kind="ExternalOutput")

    with tile.TileContext(nc) as tc:
        with (
            tc.tile_pool(name="consts", bufs=1) as consts,
            tc.tile_pool(name="wpool", bufs=1) as wpool,
            tc.tile_pool(name="stage", bufs=1) as stage,
            tc.tile_pool(name="gelu", bufs=1) as gelu,
            tc.tile_pool(name="qkv", bufs=1) as qkv,
            tc.tile_pool(name="attp", bufs=1) as attp,
            tc.tile_pool(name="ps", bufs=1, space="PSUM") as ps,
            tc.tile_pool(name="rowp", bufs=1) as rowp,
            tc.tile_pool(name="bcp", bufs=1) as bcp,
            tc.tile_pool(name="dramp", bufs=2, space="DRAM") as dramp,
        ):
            pools = (consts, wpool, stage, gelu, qkv, attp, ps, rowp, bcp,
                     dramp)
            cx = Ctx(nc, tc, pools)
            if repeat == 1:
                _build_body(cx, phases)
            else:
                with tc.For_i(0, repeat, 1,
                              hint_engines=(mybir.EngineType.PE,
                                            mybir.EngineType.Activation,
                                            mybir.EngineType.DVE)):
                    _build_body(cx, phases)
    nc.finalize()
    return nc


_NC_CACHE = {}


def _get_nc(repeat=1, phases=4):
    key = (repeat, phases)
    if key not in _NC_CACHE:
        _NC_CACHE[key] = build_nc(repeat, phases)
    return _NC_CACHE[key]


def kernel(**inputs: np.ndarray) -> np.ndarray:
    nc = _get_nc()
    x_in = np.ascontiguousarray(inputs["x_in"], dtype=np.float32)
    shared = {}
    for name in W_NAMES + B_NAMES:
        shared[name] = np.ascontiguousarray(inputs[name], dtype=np.float32)
    in_maps = [dict(shared, x_in=x_in[b]) for b in range(NCORES)]
    res = run_bass_kernel_spmd(nc, in_maps, core_ids=list(range(NCORES)))
    return np.stack([res.results[b]["out"] for b in range(NCORES)], axis=0)



# revision 20
# speedup vs baseline: 4.4470x; 4.4470x over previous
"""Trainium2 Bass kernel for nn_DilatedAttentionBlock_attention.

Per-core work (data-parallel over batch, 8 cores):
  x [C=256, L=2048] -> QKV MLPs -> 4-head attention with Lipschitz score
  rescale -> out-proj -> LayerNorm -> ELU + residual -> out [C, L].

Everything stays in channel-major ("transposed") [C, L] layout, which is the
native layout of x_in, so weights act as natural lhsT operands and no input
or output transposes are needed.  Scores are computed directly transposed
(S^T[k, q]) so the softmax exp on the scalar engine doubles as the
PSUM->SBUF copy and the attention matrix never needs transposing for AV.

The schedule is built around the Activation engine being the bottleneck
(the 128 softmax exps of [128, 1024] are ~133us of ACT busy time, more
than the PE's total matmul work).  Projections run K -> Q -> V with
interleaved emission so ct0's alpha (Lipschitz scale) is ready early and
the exp stream starts while V's second linear / transposes still run on
PE/DVE.  Row broadcasts use gpsimd (Pool) partition_broadcast or PE
rank-1 matmuls instead of DRAM DMA bounces.  Tails are emitted
mid-quarter-stream with a tiny ACT footprint; the final tail folds the
LayerNorm affine into PE rank-1 broadcasts (g (x) rstd, g (x) mrs - b)
and uses ACT Square straight out of PSUM for the z^2 moment.

Key algebraic tricks (exact, up to float rounding):
  - row_norm^2[q] = Q[q]^T (K^T K) Q[q] via a tiny 64x64 Gram matrix;
    alpha[q] = 1/sqrt(t[q]) is folded into Q before the score matmul.
  - softmax denominator: V gets a ones-column appended (M=65 AV matmul),
    so row 64 of the AV accumulator is sum_k exp(s).
  - elu: exp(min(u,0)) == min(exp(u), 1) (monotonicity), saving one op.
Matmuls run in float32r / bf16 (full PE column rate) with fp32 PSUM.

SBUF note: tags are heavily cross-phase reused to stay inside the
192KB/partition budget: k_t lives in x's staging slot, v_t in g_q's,
qt in g_k's, y/z in the k/q projection slots, ELU temps in g_v's.
"""

import numpy as np

import concourse.bacc as bacc
import concourse.bass as bass
import concourse.mybir as mybir
import concourse.tile as tile
from concourse.bass_utils import run_bass_kernel_spmd
from concourse.masks import make_identity

B, C, L, H, HD = 8, 256, 2048, 4, 64
P = 128
NCORES = 8
LH = L // 2  # 1024, attention q-half width
FP32 = mybir.dt.float32
FP32R = mybir.dt.float32r
AF = mybir.ActivationFunctionType
OP = mybir.AluOpType

W_NAMES = ["q_w1", "q_w2", "k_w1", "k_w2", "v_w1", "v_w2", "o_w"]
B_NAMES = ["q_b1", "q_b2", "k_b1", "k_b2", "v_b1", "v_b2", "o_b", "ln_g", "ln_b"]

LN_EPS = 1e-5
INV_C = 1.0 / C
BF16 = mybir.dt.bfloat16
SDT = BF16
NMM = 512

PS_TAGS = ["pA0", "pA1", "pB0", "pB1"]


class Ctx:
    """Holds pools + round-robin psum tag allocation."""

    def __init__(self, nc, tc, pools):
        self.nc = nc
        self.tc = tc
        (self.consts, self.wpool, self.stage, self.gelu, self.qkv, self.attp,
         self.ps, self.rowp, self.bcp) = pools
        self._ps_i = 0

    def ps_tile(self, shape, name):
        tag = PS_TAGS[self._ps_i % 4]
        self._ps_i += 1
        return self.ps.tile(shape, FP32, tag=tag, name=name, bufs=1)


def _linear_T(cx, w_sb, rhs_tiles, out_tiles, act_fn, bias_sb):
    """out^T[m, l] = act(sum_k w[k, m] * rhs^T[k, l] + bias[m]).

    w_sb [P, 2, C] fp32r; rhs_tiles: 2 tiles [P, L] fp32r (contraction
    k-outer); out_tiles: 2 tiles [P, L].  PSUM in [P, LH] chunks.
    """
    nc = cx.nc
    for m in range(2):
        for lh in range(2):
            acc = cx.ps_tile([P, LH], f"lin_acc_{m}_{lh}")
            for lg in range(2):
                psl = slice(lg * 512, (lg + 1) * 512)
                gsl = slice(lh * LH + lg * 512, lh * LH + (lg + 1) * 512)
                for k in range(2):
                    nc.tensor.matmul(
                        acc[:, psl],
                        w_sb[:, k, m * P : (m + 1) * P],
                        rhs_tiles[k][:, gsl],
                        start=(k == 0),
                        stop=(k == 1),
                    )
            osl = slice(lh * LH, (lh + 1) * LH)
            if act_fn is not None:
                cx.nc.scalar.activation(
                    out_tiles[m][:, osl], acc[:], act_fn,
                    bias=bias_sb[:, m : m + 1],
                )
            else:
                nc.vector.tensor_scalar_add(
                    out_tiles[m][:, osl], acc[:], bias_sb[:, m : m + 1]
                )


def _build_body(cx, phases=4):
    nc = cx.nc
    x_in, out = nc.x_in_t, nc.out_t

    def _dump(tiles):
        for m, t in enumerate(tiles):
            v = t.bitcast(FP32)
            nc.sync.dma_start(out[m * P : (m + 1) * P, 0 : v.shape[-1]], v[:])

    # ---- biases as [P, 2] columns (swdge path, off the SP queue, issued
    # first so the Pool queue serves them before the weight rounds);
    # ln rows as [1, C] for rank-1 bcast ----
    b_sb = {}
    for name in B_NAMES:
        t = cx.consts.tile([P, 2], FP32, name="b_" + name)
        nc.gpsimd.dma_start(
            t[:], getattr(nc, name + "_t").rearrange("(mo mi) -> mi mo", mi=P)
        )
        b_sb[name] = t
    gb_row_st = cx.consts.tile([1, C], FP32, name="gb_row_st")
    nc.gpsimd.dma_start(gb_row_st[:],
                        nc.ln_g_t.rearrange("(o c) -> o c", o=1))
    g_row = cx.consts.tile([1, C], FP32R, name="g_row")
    b_row = cx.consts.tile([1, C], FP32R, name="b_row")

    # ---- constants (on the otherwise-idle Pool engine) ----
    ident_st = cx.consts.tile([P, P], FP32)
    make_identity(nc, ident_st)
    ident = cx.consts.tile([P, P], FP32R)
    nc.vector.tensor_copy(ident[:], ident_st[:])
    ones_st = cx.consts.tile([P, 32], FP32)
    nc.gpsimd.memset(ones_st[:], 1.0)
    invc_st = cx.consts.tile([P, 1], FP32)
    nc.gpsimd.memset(invc_st[:], INV_C)
    invc_128 = cx.consts.tile([P, 1], FP32R)
    nc.vector.tensor_copy(invc_128[:], invc_st[:])
    eps_sb = cx.consts.tile([1, 1], FP32)
    nc.gpsimd.memset(eps_sb[:], LN_EPS)
    mones_st = cx.rowp.tile([1, LH], FP32, tag="lnt", name="mones_st",
                            bufs=1)
    nc.gpsimd.memset(mones_st[:], -1.0)
    mones_row = cx.consts.tile([1, LH], FP32R)
    nc.vector.tensor_copy(mones_row[:], mones_st[:])
    # sel2 [P, 2]: column h selects head-h's 64 partitions (for t row sums)
    sel_st = cx.consts.tile([P, 2], FP32)
    nc.gpsimd.memset(sel_st[:], 0.0)
    nc.gpsimd.memset(sel_st[0:64, 0:1], 1.0)
    nc.gpsimd.memset(sel_st[64:128, 1:2], 1.0)
    sel2 = cx.consts.tile([P, 2], FP32R)
    nc.vector.tensor_copy(sel2[:], sel_st[:])
    # selbc [2, P]: row r broadcasts to partition group r (alpha broadcast).
    # Partition-1-only writes are illegal for memset, so build it as the
    # PE transpose of sel2.
    selbc = cx.consts.tile([2, P], FP32R)
    trsel = cx.ps_tile([2, P], "trsel")
    nc.tensor.transpose(trsel.bitcast(FP32R)[:], sel2[:], ident[:])
    nc.vector.tensor_copy(selbc[:], trsel.bitcast(FP32R)[:])
    gz_st = cx.consts.tile([P, P], FP32)
    nc.gpsimd.memset(gz_st[:], 0.0)
    nc.vector.tensor_copy(g_row[:], gb_row_st[:])
    nc.gpsimd.dma_start(gb_row_st[:],
                        nc.ln_b_t.rearrange("(o c) -> o c", o=1))
    nc.vector.tensor_copy(b_row[:], gb_row_st[:])

    # ---- first-needed weight, then x (l-chunked), then the rest; the SP
    # DMA queue issues in order at ~565ns each, so order = priority ----
    def load_w(name, tag):
        st = cx.wpool.tile([P, 2, C], FP32, tag="w_stage", bufs=1,
                           name=f"wst_{name}")
        nc.sync.dma_start(
            st[:],
            getattr(nc, name + "_t").rearrange("(ko ki) m -> ki ko m", ki=P),
        )
        wr = cx.wpool.tile([P, 2, C], FP32R, tag=tag, name=f"w_{name}", bufs=1)
        nc.vector.tensor_copy(wr[:], st[:])
        return wr

    w_k1 = load_w("k_w1", "wA")

    x_re = x_in.rearrange("(ko ki) l -> ki ko l", ki=P)
    xst = cx.stage.tile([P, 2, L], FP32, tag="x_st")
    xr = cx.stage.tile([P, 2, L], FP32R, tag="xr")
    NXC = 4
    XC = L // NXC
    for c in range(NXC):
        xsl = slice(c * XC, (c + 1) * XC)
        nc.sync.dma_start(xst[:, :, xsl], x_re[:, :, xsl])
        nc.vector.tensor_copy(xr[:, :, xsl], xst[:, :, xsl])

    w_q1 = load_w("q_w1", "wB")
    w_k2 = load_w("k_w2", "wC")
    w_v1 = load_w("v_w1", "wD")
    w_q2 = load_w("q_w2", "wE")
    w_v2 = load_w("v_w2", "wF")
    ow_sb = load_w("o_w", "w_ow")

    # ---- MLPs, K -> Q -> (V.w1+gelu), interleaved on the PE queue ----
    g_k = [cx.gelu.tile([P, L], FP32R, tag=f"gk{m}", name=f"g_k{m}", bufs=1)
           for m in range(2)]
    g_q = [cx.gelu.tile([P, L], FP32R, tag=f"gq{m}", name=f"g_q{m}", bufs=1)
           for m in range(2)]
    g_v = [cx.gelu.tile([P, L], FP32R, tag=f"gv{m}", name=f"g_v{m}", bufs=1)
           for m in range(2)]
    k_sb = [cx.qkv.tile([P, L], FP32R, tag=f"k{m}", name=f"k_sb{m}", bufs=1)
            for m in range(2)]
    q_sb = [cx.qkv.tile([P, L], FP32R, tag=f"q{m}", name=f"q_sb{m}", bufs=1)
            for m in range(2)]
    v_sb = [cx.qkv.tile([P, L], FP32R, tag=f"v{m}", name=f"v_sb{m}", bufs=1)
            for m in range(2)]

    _linear_T(cx, w_k1, [xr[:, 0], xr[:, 1]], g_k, AF.Gelu, b_sb["k_b1"])
    _linear_T(cx, w_q1, [xr[:, 0], xr[:, 1]], g_q, AF.Gelu, b_sb["q_b1"])
    _linear_T(cx, w_k2, g_k, k_sb, None, b_sb["k_b2"])
    _linear_T(cx, w_v1, [xr[:, 0], xr[:, 1]], g_v, AF.Gelu, b_sb["v_b1"])

    # kb: bf16 K for the score matmuls (both ct in one tile)
    kb = cx.qkv.tile([P, 2, L], SDT, tag="kb", name="kb", bufs=1)
    for ct in range(2):
        nc.vector.tensor_copy(kb[:, ct, :], k_sb[ct][:])

    # ---- K transposes + Gram (PE); k_t reuses x's staging slot and is
    # single-buffered (gram ct0 completes before ktrans ct1 overwrites) ----
    g_pair = []
    for ct in range(2):
        kt_tile = cx.stage.tile([P, 16, P], FP32R, tag="x_st",
                                name=f"k_t{ct}", bufs=1)
        for lt0 in range(0, 16, 4):
            trk = cx.ps_tile([P, 512], f"trk_{ct}_{lt0}")
            for j in range(4):
                nc.tensor.transpose(
                    trk.bitcast(FP32R)[:, j * P : (j + 1) * P],
                    k_sb[ct][:, (lt0 + j) * P : (lt0 + j + 1) * P],
                    ident[:],
                )
            nc.vector.tensor_copy(
                kt_tile[:, lt0 : lt0 + 4, :],
                trk.bitcast(FP32R).rearrange("p (l c) -> p l c", l=4),
            )
        g_ps = cx.ps_tile([P, P], f"g_ps{ct}")
        for kt in range(16):
            nc.tensor.matmul(
                g_ps[:], kt_tile[:, kt, :], kt_tile[:, kt, :],
                start=(kt == 0), stop=(kt == 15),
            )
        gp = cx.rowp.tile([P, P], FP32R, tag=f"gram{ct}", name=f"g_pair{ct}",
                          bufs=1)
        nc.vector.tensor_copy(gp[:], gz_st[:])
        for ho in range(2):
            hsl = slice(64 * ho, 64 * ho + 64)
            nc.vector.tensor_copy(gp[hsl, hsl], g_ps.bitcast(FP32R)[hsl, hsl])
        g_pair.append(gp)

    _linear_T(cx, w_q2, g_q, q_sb, None, b_sb["q_b2"])

    # ---- alpha(ct) -> qt(ct): GQ, t, ln, exp, PE rank-1 broadcast ----
    qt_sb = [cx.gelu.tile([P, L], SDT, tag=f"gk{ct}", name=f"qt{ct}", bufs=1)
             for ct in range(2)]

    def alpha_qt(ct):
        lnt = cx.rowp.tile([2, L], FP32R, tag="lnt", name=f"lnt{ct}", bufs=1)
        for lh in range(2):
            lsl = slice(lh * LH, (lh + 1) * LH)
            qgq = cx.rowp.tile([P, LH], FP32R, tag="qgq", name=f"qgq{ct}{lh}",
                               bufs=1)
            gq_ps = cx.ps_tile([P, LH], f"gq_ps{ct}{lh}")
            for lg in range(2):
                psl = slice(lg * 512, (lg + 1) * 512)
                gsl = slice(lh * LH + lg * 512, lh * LH + (lg + 1) * 512)
                nc.tensor.matmul(gq_ps[:, psl], g_pair[ct][:], q_sb[ct][:, gsl],
                                 start=True, stop=True)
            nc.vector.tensor_tensor(out=qgq[:], in0=q_sb[ct][:, lsl],
                                    in1=gq_ps[:], op=OP.mult)
            t_ps = cx.ps_tile([2, LH], f"t_ps{ct}{lh}")
            for lg in range(2):
                psl = slice(lg * 512, (lg + 1) * 512)
                nc.tensor.matmul(t_ps[:, psl], sel2[:], qgq[:, psl],
                                 start=True, stop=True)
            nc.scalar.activation(lnt[:, lsl], t_ps[:], AF.Ln)
        # in-place exp: lnt becomes alpha = exp(-0.5 ln t) = 1/sqrt(t)
        nc.scalar.activation(lnt[:], lnt[:], AF.Exp, scale=-0.5)
        # broadcast row h to partition group h via PE, multiply into qt
        for lh in range(2):
            lsl = slice(lh * LH, (lh + 1) * LH)
            abc_ps = cx.ps_tile([P, LH], f"abc{ct}{lh}")
            for lg in range(2):
                psl = slice(lg * 512, (lg + 1) * 512)
                nc.tensor.matmul(
                    abc_ps[:, psl], selbc[:],
                    lnt[:, lh * LH + lg * 512 : lh * LH + (lg + 1) * 512],
                    start=True, stop=True)
            nc.vector.tensor_tensor(out=qt_sb[ct][:, lsl],
                                    in0=q_sb[ct][:, lsl], in1=abc_ps[:],
                                    op=OP.mult)

    alpha_qt(0)

    _linear_T(cx, w_v2, g_v, v_sb, None, b_sb["v_b2"])

    # ---- V transposes (ones-augmented); v_t reuses g_q's slots ----
    v_t = []
    for ct in range(2):
        vt_tile = cx.gelu.tile([P, 16, 130], SDT, tag=f"gq{ct}",
                               name=f"v_t{ct}", bufs=1)
        nc.vector.tensor_copy(
            vt_tile.rearrange("p l (h c) -> p l h c", h=2)[:, :, :, 64:65],
            ones_st.rearrange("p (l h c) -> p l h c", l=16, h=2),
        )
        for lt0 in range(0, 16, 4):
            trv = cx.ps_tile([P, 512], f"trv_{ct}_{lt0}")
            for j in range(4):
                nc.tensor.transpose(
                    trv.bitcast(FP32R)[:, j * P : (j + 1) * P],
                    v_sb[ct][:, (lt0 + j) * P : (lt0 + j + 1) * P],
                    ident[:],
                )
            nc.vector.tensor_copy(
                vt_tile[:, lt0 : lt0 + 4, :]
                .rearrange("p l (h c) -> p l h c", h=2)[:, :, :, 0:64],
                trv.bitcast(FP32R).rearrange("p (l h c) -> p l h c", l=4, h=2),
            )
        v_t.append(vt_tile)

    alpha_qt(1)

    if phases == 1:
        _dump(q_sb)
        return
    if phases == 2:
        _dump(qt_sb)
        return

    # ---- attention + per-half tails (y/z reuse the k/q slots) ----
    y_sb = [cx.qkv.tile([P, L], FP32R, tag=f"k{ct}", name=f"y{ct}", bufs=1)
            for ct in range(2)]
    z_sb = [cx.qkv.tile([P, L], FP32R, tag=f"q{m}", name=f"z{m}", bufs=1)
            for m in range(2)]

    def quarter(qh, ct, ho):
        """One head, one q-half.  Score accumulator double-buffered on kt
        parity; AV lags one kt so it is never exp-gated at the head of the
        in-order PE queue."""
        q0 = qh * LH
        hslice = slice(q0, q0 + LH)
        hsl = slice(64 * ho, 64 * ho + 64)
        b_ps = cx.ps.tile([65, LH], FP32, tag=PS_TAGS[2 + ho],
                          name=f"av{ct}{qh}{ho}", bufs=1)

        def s_mm(kt):
            a = cx.ps.tile([P, LH], FP32, tag=PS_TAGS[kt % 2],
                           name=f"s{ct}{qh}{kt}{ho}", bufs=1)
            for lg in range(LH // NMM):
                psl = slice(lg * NMM, (lg + 1) * NMM)
                nc.tensor.matmul(
                    a[:, psl],
                    kb[hsl, ct, kt * P : (kt + 1) * P],
                    qt_sb[ct][hsl, q0 + lg * NMM : q0 + (lg + 1) * NMM],
                    start=True, stop=True,
                )
            return a

        def av_mm(kt, attn):
            for lg in range(LH // NMM):
                psl = slice(lg * NMM, (lg + 1) * NMM)
                nc.tensor.matmul(
                    b_ps[:, psl],
                    v_t[ct][:, kt, 65 * ho : 65 * ho + 65],
                    attn[:, psl],
                    start=(kt == 0), stop=(kt == 15),
                )

        a_cur = s_mm(0)
        attn_prev = None
        for kt in range(16):
            attn = cx.attp.tile([P, LH], SDT, tag=f"attn{kt % 2}",
                                name=f"at{ct}{qh}{kt}{ho}", bufs=2)
            nc.scalar.activation(attn[:], a_cur[:], AF.Exp)
            if kt < 15:
                a_cur = s_mm(kt + 1)
            if attn_prev is not None:
                av_mm(kt - 1, attn_prev)
            attn_prev = attn
        av_mm(15, attn_prev)

        # drain: yc out of PSUM promptly; 1/d broadcast on the Pool engine
        invd = cx.rowp.tile([1, LH], FP32, tag="rowA",
                            name=f"invd{ct}{qh}{ho}", bufs=1)
        nc.vector.reciprocal(invd[:], b_ps[64:65, :])
        yc = cx.rowp.tile([64, LH], FP32,
                          tag=("qgq" if ho == 0 else "z2b"),
                          name=f"yc{qh}{ct}{ho}", bufs=1)
        nc.vector.tensor_copy(yc[:], b_ps[0:64, :])
        dbc = cx.bcp.tile([64, LH], FP32, tag="bc",
                          name=f"dbc{ct}{qh}{ho}", bufs=2)
        nc.gpsimd.partition_broadcast(dbc[:], invd[:])
        nc.vector.tensor_tensor(
            out=y_sb[ct][hsl, hslice], in0=yc[:], in1=dbc[:], op=OP.mult,
        )

    def half_tail(qh, final):
        q0 = qh * LH
        hslice = slice(q0, q0 + LH)
        # out-proj into PSUM (pB tags; ordered behind this point's quarters)
        zps = []
        for m in range(2):
            acc = cx.ps.tile([P, LH], FP32, tag=PS_TAGS[2 + m],
                             name=f"zacc{qh}{m}", bufs=1)
            for lg in range(2):
                psl = slice(lg * 512, (lg + 1) * 512)
                gsl = slice(q0 + lg * 512, q0 + (lg + 1) * 512)
                for ct in range(2):
                    nc.tensor.matmul(
                        acc[:, psl],
                        ow_sb[:, ct, m * P : (m + 1) * P],
                        y_sb[ct][:, gsl],
                        start=(ct == 0), stop=(ct == 1),
                    )
            zps.append(acc)
        # drains (DVE/Pool split) + z^2
        z2h = [cx.rowp.tile([P, LH], FP32R,
                            tag=("qgq" if m == 0 else "z2b"),
                            name=f"z2_{qh}{m}", bufs=1) for m in range(2)]
        for m in range(2):
            nc.vector.tensor_scalar_add(z_sb[m][:, hslice], zps[m][:],
                                        b_sb["o_b"][:, m : m + 1])
        if final:
            # ACT is idle in the suffix: square straight out of PSUM
            for m in range(2):
                nc.scalar.activation(z2h[m][:], zps[m][:], AF.Square,
                                     bias=b_sb["o_b"][:, m : m + 1])
        else:
            for m in range(2):
                nc.vector.tensor_tensor(out=z2h[m][:], in0=z_sb[m][:, hslice],
                                        in1=z_sb[m][:, hslice], op=OP.mult)
        s1_ps = cx.ps.tile([1, LH], FP32, tag=PS_TAGS[2], name=f"s1_{qh}",
                           bufs=1)
        s2_ps = cx.ps.tile([1, LH], FP32, tag=PS_TAGS[3], name=f"s2_{qh}",
                           bufs=1)
        for lg in range(2):
            psl = slice(lg * 512, (lg + 1) * 512)
            gsl = slice(q0 + lg * 512, q0 + (lg + 1) * 512)
            for m in range(2):
                nc.tensor.matmul(s1_ps[:, psl], invc_128[:], z_sb[m][:, gsl],
                                 start=(m == 0), stop=(m == 1))
            for m in range(2):
                nc.tensor.matmul(s2_ps[:, psl], invc_128[:], z2h[m][:, psl],
                                 start=(m == 0), stop=(m == 1))
        # mean/var rows ([1, LH]); the rstd tile is reused in place
        # through var2 -> ln -> rstd
        mu = cx.rowp.tile([1, LH], FP32, tag="rowB", name=f"mu{qh}", bufs=1)
        nc.vector.tensor_copy(mu[:], s1_ps[:])
        var = cx.rowp.tile([1, LH], FP32, tag="lnt", name=f"var{qh}", bufs=1)
        nc.vector.tensor_tensor(out=var[:], in0=mu[:], in1=mu[:], op=OP.mult)
        var2 = cx.rowp.tile([1, LH], FP32, tag="z2b", name=f"var2{qh}",
                            bufs=1)
        nc.vector.scalar_tensor_tensor(out=var2[:], in0=s2_ps[:], scalar=0.0,
                                       in1=var[:], op0=OP.add,
                                       op1=OP.subtract)
        nc.scalar.activation(var2[:], var2[:], AF.Ln, bias=eps_sb[:])
        rstd = cx.rowp.tile([1, LH], FP32R, tag="rowC", name=f"rstd{qh}",
                            bufs=1)
        nc.scalar.activation(rstd[:], var2[:], AF.Exp, scale=-0.5)
        mrs = cx.rowp.tile([1, LH], FP32R, tag="rowA", name=f"mrs{qh}",
                           bufs=1)
        nc.vector.tensor_tensor(out=mrs[:], in0=mu[:], in1=rstd[:],
                                op=OP.mult)

        if final:
            # rank-1 PE broadcasts: A = g (x) rstd, Cm = g (x) mrs - b (x) 1
            # (PSUM is free now); then u = z*A - Cm.
            for m in range(2):
                a_ps = cx.ps.tile([P, LH], FP32, tag=PS_TAGS[m],
                                  name=f"Abc{qh}{m}", bufs=1)
                c_ps = cx.ps.tile([P, LH], FP32, tag=PS_TAGS[2 + m],
                                  name=f"Cbc{qh}{m}", bufs=1)
                for lg in range(2):
                    psl = slice(lg * 512, (lg + 1) * 512)
                    nc.tensor.matmul(a_ps[:, psl],
                                     g_row[:, m * P : (m + 1) * P],
                                     rstd[:, psl], start=True, stop=True)
                    nc.tensor.matmul(c_ps[:, psl],
                                     g_row[:, m * P : (m + 1) * P],
                                     mrs[:, psl], start=True, stop=False)
                    nc.tensor.matmul(c_ps[:, psl],
                                     b_row[:, m * P : (m + 1) * P],
                                     mones_row[:, psl], start=False, stop=True)
                u = y_sb[m][:, hslice]
                nc.vector.tensor_tensor(out=u[:], in0=z_sb[m][:, hslice],
                                        in1=a_ps[:], op=OP.mult)
                nc.vector.tensor_tensor(out=u[:], in0=u[:], in1=c_ps[:],
                                        op=OP.subtract)
        else:
            # Pool partition_broadcast for rstd/mrs; 3-op affine split
            # between DVE and Pool per m
            rstdb = cx.bcp.tile([P, LH], FP32, tag="bc", name=f"rstdb{qh}",
                                bufs=2)
            nc.gpsimd.partition_broadcast(rstdb[:], rstd[:].bitcast(FP32))
            mrsb = cx.bcp.tile([P, LH], FP32, tag="bc", name=f"mrsb{qh}",
                               bufs=2)
            nc.gpsimd.partition_broadcast(mrsb[:], mrs[:].bitcast(FP32))
            for m in range(2):
                u = y_sb[m][:, hslice]
                nc.vector.tensor_tensor(out=u[:], in0=z_sb[m][:, hslice],
                                        in1=rstdb[:], op=OP.mult)
                nc.vector.tensor_tensor(out=u[:], in0=u[:], in1=mrsb[:],
                                        op=OP.subtract)
                nc.vector.tensor_scalar(
                    out=u[:], in0=u[:],
                    scalar1=b_sb["ln_g"][:, m : m + 1],
                    scalar2=b_sb["ln_b"][:, m : m + 1],
                    op0=OP.mult, op1=OP.add,
                )

        # elu(u) + x = max(u,0) + min(exp(u),1) - 1 + x
        for m in range(2):
            u = y_sb[m][:, hslice]
            e = cx.gelu.tile([P, LH], FP32, tag=f"gv{m}", name=f"e{qh}{m}",
                             bufs=1)
            nc.scalar.activation(e[:], u[:], AF.Exp)
            nc.vector.tensor_scalar_min(e[:], e[:], 1.0)
            nc.vector.scalar_tensor_tensor(out=u[:], in0=u[:], scalar=0.0,
                                           in1=e[:], op0=OP.max, op1=OP.add)
            nc.vector.scalar_tensor_tensor(out=u[:], in0=u[:], scalar=-1.0,
                                           in1=xr[:, m, hslice], op0=OP.add,
                                           op1=OP.add)
            nc.gpsimd.dma_start(out[m * P : (m + 1) * P, hslice],
                              u.bitcast(FP32)[:])

    if phases == 3:
        for qh in range(2):
            for ct in range(2):
                for ho in range(2):
                    quarter(qh, ct, ho)
        _dump(y_sb)
        return

    for ct in range(2):
        for ho in range(2):
            quarter(0, ct, ho)
    quarter(1, 0, 0)
    quarter(1, 0, 1)
    half_tail(0, final=False)
    quarter(1, 1, 0)
    quarter(1, 1, 1)
    half_tail(1, final=True)


def _steer_act_tables():
    """The act-table-load pass picks the first set containing each
    function, which thrashes natural_log <-> exp_and_others when a kernel
    uses both Ln and Exp.  Empty out the single-function sets so both
    resolve to natural_log_exp_and_others (ids keep their positions)."""
    import concourse.hw_specs as hw_specs
    if getattr(hw_specs, "_act_tables_steered", False):
        return
    orig = hw_specs.get_activation_tables

    def patched(arch):
        t = dict(orig(arch))
        for k in ("natural_log", "exp_and_others", "exp_and_friends"):
            if k in t:
                t[k] = set()
        return t

    hw_specs.get_activation_tables = patched
    bacc.get_activation_tables = patched
    hw_specs._act_tables_steered = True


def build_nc(repeat: int = 1, phases: int = 4):
    _steer_act_tables()
    nc = bacc.Bacc("TRN2", target_bir_lowering=False)
    nc.x_in_t = nc.dram_tensor("x_in", [C, L], FP32, kind="ExternalInput")
    for name in W_NAMES:
        setattr(nc, name + "_t",
                nc.dram_tensor(name, [C, C], FP32, kind="ExternalInput"))
    for name in B_NAMES:
        setattr(nc, name + "_t",
                nc.dram_tensor(name, [C], FP32, kind="ExternalInput"))
    nc.out_t = nc.dram_tensor("out", [C, L], FP32, kind="ExternalOutput")

    with tile.TileContext(nc) as tc:
        with (
            tc.tile_pool(name="consts", bufs=1) as consts,
            tc.tile_pool(name="wpool", bufs=1) as wpool,
            tc.tile_pool(name="stage", bufs=1) as stage,
            tc.tile_pool(name="gelu", bufs=1) as gelu,
            tc.tile_pool(name="qkv", bufs=1) as qkv,
            tc.tile_pool(name="attp", bufs=1) as attp,
            tc.tile_pool(name="ps", bufs=1, space="PSUM") as ps,
            tc.tile_pool(name="rowp", bufs=1) as rowp,
            tc.tile_pool(name="bcp", bufs=1) as bcp,
        ):
            pools = (consts, wpool, stage, gelu, qkv, attp, ps, rowp, bcp)
            cx = Ctx(nc, tc, pools)
            if repeat == 1:
                _build_body(cx, phases)
            else:
                with tc.For_i(0, repeat, 1,
                              hint_engines=(mybir.EngineType.PE,
                                            mybir.EngineType.Activation,
                                            mybir.EngineType.DVE,
                                            mybir.EngineType.Pool)):
                    _build_body(cx, phases)
    nc.finalize()
    return nc


_NC_CACHE = {}


def _get_nc(repeat=1, phases=4):
    key = (repeat, phases)
    if key not in _NC_CACHE:
        _NC_CACHE[key] = build_nc(repeat, phases)
    return _NC_CACHE[key]


def kernel(**inputs: np.ndarray) -> np.ndarray:
    nc = _get_nc()
    x_in = np.ascontiguousarray(inputs["x_in"], dtype=np.float32)
    shared = {}
    for name in W_NAMES + B_NAMES:
        shared[name] = np.ascontiguousarray(inputs[name], dtype=np.float32)
    in_maps = [dict(shared, x_in=x_in[b]) for b in range(NCORES)]
    res = run_bass_kernel_spmd(nc, in_maps, core_ids=list(range(NCORES)))
    return np.stack([res.results[b]["out"] for b in range(NCORES)], axis=0)
